# revision 9
# baseline (speedup 1.0000x reference)
"""Trainium2 Bass kernel for nn_AmplifierAttention (sparse sigmoid-threshold attention).

  t    = text @ W^T + b          [S, E]
  attn = t @ L^T                 [S, Lb]
  a    = sigmoid(attn); a[a < 0.4] = 0
  out  = softmax(a, axis=-1) @ L [S, E]

Strategy: data-parallel over batch B=8 -> one batch element per NeuronCore,
weights replicated, zero collectives.  Per core, everything is computed
transposed (contraction dims on partitions) so no on-chip transposes are
needed; the host pre-transposes text/W/L instead.

Softmax trick: softmax(a) @ L == (exp(a - c) @ L) / rowsum(exp(a - c)) for any
constant c.  With h = tanh(attn/2) (same ACT table set as exp), sigmoid =
(h+1)/2 and the thresholded exponent is exp(0.5*u - 0.5) where
u = (h+1)*[h >= -0.2]  (sigmoid(x) < 0.4  <=>  tanh(x/2) < -0.2).
The rowsum comes free from a ones-column appended to L in the last matmul.
All matmul inputs bf16 (fp32 PSUM accumulation): measured rel err ~7e-3.
"""

import os
import numpy as np
import ml_dtypes

P = 128
BF = ml_dtypes.bfloat16

_PROGRAM_CACHE = {}


def build_program(S=2048, DT=1024, E=768, L=4096, SC=512):
    """Build + compile the per-core Bass program (same SPMD program on all cores)."""
    from contextlib import ExitStack
    import concourse.bass as bass  # noqa: F401
    import concourse.mybir as mybir
    import concourse.tile as tile
    from concourse import bacc

    dt = mybir.dt
    AF = mybir.ActivationFunctionType
    OP = mybir.AluOpType

    ND = DT // P        # d-tiles
    NE = E // P         # e-tiles
    NL = L // P         # l-tiles
    NPAIR = NL // 2     # l-pairs (two l-tiles share one 2-bank PSUM tile)
    NCH = S // SC       # s-chunks
    NSS = SC // P       # s-subtiles per chunk
    EH = E // 2         # half of the output feature dim
    EP = E + 2          # padded label row: E features + ones col + zero pad

    nc = bacc.Bacc("TRN2", target_bir_lowering=False, debug=False)

    tt = nc.dram_tensor("tt", [DT, S], dt.bfloat16, kind="ExternalInput").ap()
    wt = nc.dram_tensor("wt", [DT, E], dt.bfloat16, kind="ExternalInput").ap()
    lt = nc.dram_tensor("lt", [E, L], dt.bfloat16, kind="ExternalInput").ap()
    laug = nc.dram_tensor("laug", [L, EP], dt.bfloat16, kind="ExternalInput").ap()
    bb = nc.dram_tensor("bb", [E], dt.float32, kind="ExternalInput").ap()
    out = nc.dram_tensor("out", [S, E], dt.float32, kind="ExternalOutput").ap()

    with tile.TileContext(nc) as tc, ExitStack() as ctx:
        const_pool = ctx.enter_context(tc.tile_pool(name="const", bufs=1))
        tt_pool = ctx.enter_context(tc.tile_pool(name="ttp", bufs=1))
        t_pool = ctx.enter_context(tc.tile_pool(name="tp", bufs=1))
        w_pool = ctx.enter_context(tc.tile_pool(name="wp", bufs=1))
        ew_pool = ctx.enter_context(tc.tile_pool(name="ewp", bufs=2))
        o_pool = ctx.enter_context(tc.tile_pool(name="op", bufs=1))
        r_pool = ctx.enter_context(tc.tile_pool(name="rp", bufs=2))
        pa_pool = ctx.enter_context(tc.tile_pool(name="pap", bufs=2, space="PSUM"))
        po_pool = ctx.enter_context(tc.tile_pool(name="pop", bufs=4, space="PSUM"))

        # --- resident weights.  Everything rides the sync HWDGE ring, which
        # is FIFO per issuing engine — so emission order IS bandwidth
        # priority: step-1 critical tensors (wt, chunk-0 text) first, then
        # the label tensors (first needed ~20us / ~60us in).
        wt_r = wt.rearrange("(a p) e -> p a e", p=P)
        wt_sb = const_pool.tile([P, ND, E], dt.bfloat16, tag="wt")
        # the e=0 stationary tiles land first so the first LDWEIGHTS can
        # issue ~1us in; chunk-0 text streams per d-tile right behind them
        nc.sync.dma_start(wt_sb[:, :, 0:P], wt_r[:, :, 0:P])
        tt_r = tt.rearrange("(a p) s -> p a s", p=P)
        tt0_sb = tt_pool.tile([P, ND, SC], dt.bfloat16, tag="tt")
        for d in range(ND):
            nc.sync.dma_start(tt0_sb[:, d, :], tt_r[:, d, 0:SC])
        nc.sync.dma_start(wt_sb[:, :, P:E], wt_r[:, :, P:E])
        b_sb = const_pool.tile([P, NE], dt.float32, tag="b")
        nc.sync.dma_start(b_sb[:], bb.rearrange("(a p) -> p a", p=P))
        nbias = const_pool.tile([P, 1], dt.float32, tag="nb")
        nc.vector.memset(nbias[:], -0.5)

        lt_sb = const_pool.tile([P, NE, L], dt.bfloat16, tag="lt")
        lt_r = lt.rearrange("(a p) l -> p a l", p=P)
        for e in range(NE):
            nc.sync.dma_start(lt_sb[:, e, :], lt_r[:, e, :])
        la_sb = const_pool.tile([P, NL, EP], dt.bfloat16, tag="la")
        la_r = laug.rearrange("(a p) e -> p a e", p=P)
        for li in range(NL):
            nc.sync.dma_start(la_sb[:, li, :], la_r[:, li, :])

        for c in range(NCH):
            s0 = c * SC
            # ---- step 1: t^T[e, s] = sum_d W^T[d,e] * text^T[d,s]  (+ bias)
            if c == 0:
                tt_sb = tt0_sb
            else:
                tt_sb = tt_next
            t_sb = t_pool.tile([P, NE, SC], dt.bfloat16, tag="t")
            for e in range(NE):
                ps = pa_pool.tile([P, 2 * SC], dt.float32, tag="pa")
                for d in range(ND):
                    nc.tensor.matmul(
                        ps[:, :SC],
                        lhsT=wt_sb[:, d, e * P:(e + 1) * P],
                        rhs=tt_sb[:, d, :],
                        start=(d == 0), stop=(d == ND - 1),
                    )
                nc.scalar.activation(t_sb[:, e, :], ps[:, :SC],
                                     AF.Identity, bias=b_sb[:, e:e + 1])
            if c + 1 < NCH:
                # prefetch next chunk's text now so its DMA sits ahead of this
                # chunk's output stores on the sync FIFO
                tt_next = tt_pool.tile([P, ND, SC], dt.bfloat16, tag="tt")
                nc.sync.dma_start(tt_next[:], tt_r[:, :, s0 + SC:s0 + 2 * SC])

            # ---- step 2: attn^T[l, s] per l-pair + elementwise -> w
            w_sb = w_pool.tile([P, NPAIR, 2 * SC], dt.bfloat16, tag="w")
            for pr in range(NPAIR):
                pa = pa_pool.tile([P, 2 * SC], dt.float32, tag="pa")
                for sub in range(2):
                    li = 2 * pr + sub
                    for e in range(NE):
                        nc.tensor.matmul(
                            pa[:, sub * SC:(sub + 1) * SC],
                            lhsT=lt_sb[:, e, li * P:(li + 1) * P],
                            rhs=t_sb[:, e, :],
                            start=(e == 0), stop=(e == NE - 1),
                        )
                h = ew_pool.tile([P, 2 * SC], dt.bfloat16, tag="h")
                nc.scalar.activation(h[:], pa[:], AF.Tanh, scale=0.5)
                hp1 = ew_pool.tile([P, 2 * SC], dt.bfloat16, tag="hp1")
                nc.vector.tensor_scalar(hp1[:], h[:], 1.0, None, OP.add)
                msk = ew_pool.tile([P, 2 * SC], dt.bfloat16, tag="m")
                nc.vector.tensor_scalar(msk[:], h[:], -0.2, None, OP.is_ge)
                u = ew_pool.tile([P, 2 * SC], dt.bfloat16, tag="u")
                nc.vector.tensor_tensor(u[:], hp1[:], msk[:], OP.mult)
                nc.scalar.activation(w_sb[:, pr, :], u[:], AF.Exp,
                                     bias=nbias[:], scale=0.5)

            # ---- step 3: out[s, :] = (w @ [L | 1]) / rowsum.  s-subtiles in
            # groups of 2; both e-halves accumulate concurrently so the two
            # matmuls sharing one stationary w-tile are adjacent (one weight
            # load).  Half 1 carries the ones column -> rowsum.
            out_sb = o_pool.tile([P, NSS, E], dt.float32, tag="osb")
            rinv = r_pool.tile([P, NSS], dt.float32, tag="rinv")
            for g0 in range(0, NSS, 2):
                sss = tuple(range(g0, min(g0 + 2, NSS)))
                pos = {(ss, hf): po_pool.tile([P, 512], dt.float32, tag="po",
                                              name=f"po_{c}_{ss}_{hf}")
                       for ss in sss for hf in (0, 1)}
                for pr in range(NPAIR):
                    for sub in range(2):
                        li = 2 * pr + sub
                        first = (li == 0)
                        last = (li == NL - 1)
                        for ss in sss:
                            lhsT = w_sb[:, pr, sub * SC + ss * P:
                                        sub * SC + (ss + 1) * P]
                            nc.tensor.matmul(
                                pos[(ss, 1)][:, :EH + 1], lhsT=lhsT,
                                rhs=la_sb[:, li, EH:E + 1],
                                start=first, stop=last,
                            )
                            nc.tensor.matmul(
                                pos[(ss, 0)][:, :EH], lhsT=lhsT,
                                rhs=la_sb[:, li, 0:EH],
                                start=first, stop=last,
                            )
                for ss in sss:
                    nc.vector.reciprocal(rinv[:, ss:ss + 1],
                                         pos[(ss, 1)][:, EH:EH + 1])
                    nc.vector.tensor_scalar(out_sb[:, ss, EH:E],
                                            pos[(ss, 1)][:, :EH],
                                            rinv[:, ss:ss + 1], None, OP.mult)
                    nc.vector.tensor_scalar(out_sb[:, ss, 0:EH],
                                            pos[(ss, 0)][:, :EH],
                                            rinv[:, ss:ss + 1], None, OP.mult)
                    nc.sync.dma_start(out[s0 + ss * P:s0 + (ss + 1) * P, :],
                                      out_sb[:, ss, :])

    nc.compile()
    return nc


def _get_program(key):
    if key not in _PROGRAM_CACHE:
        _PROGRAM_CACHE[key] = build_program(*key)
    return _PROGRAM_CACHE[key]


def prep_inputs(text_vec, labels_vec, W_proj, b_proj):
    """Host-side shard + layout prep: transpose/cast to the DRAM layouts the
    kernel expects.  Returns in_maps for run_bass_kernel_spmd."""
    B, S, DT = text_vec.shape
    L, E = labels_vec.shape
    wt = np.ascontiguousarray(W_proj.T).astype(BF)                # [DT, E]
    lt = np.ascontiguousarray(labels_vec.T).astype(BF)            # [E, L]
    laug = np.zeros((L, E + 2), dtype=BF)
    laug[:, :E] = labels_vec.astype(BF)
    laug[:, E] = 1.0
    b32 = np.ascontiguousarray(b_proj).astype(np.float32)
    in_maps = []
    for b in range(B):
        ttb = np.ascontiguousarray(text_vec[b].T).astype(BF)      # [DT, S]
        in_maps.append({"tt": ttb, "wt": wt, "lt": lt, "laug": laug, "bb": b32})
    return in_maps


def kernel(text_vec, labels_vec, W_proj, b_proj):
    from concourse.bass_utils import run_bass_kernel_spmd

    text_vec = np.asarray(text_vec)
    labels_vec = np.asarray(labels_vec)
    W_proj = np.asarray(W_proj)
    b_proj = np.asarray(b_proj)

    B, S, DT = text_vec.shape
    L, E = labels_vec.shape
    nc = _get_program((S, DT, E, L, 512))
    in_maps = prep_inputs(text_vec, labels_vec, W_proj, b_proj)

    trace = bool(int(os.environ.get("AMP_TRACE", "0")))
    res = run_bass_kernel_spmd(nc, in_maps, core_ids=list(range(B)), trace=trace)
    if trace and res.exec_time_ns is not None:
        print(f"HW exec time: {res.exec_time_ns} ns")
        if res.instructions_and_trace is not None:
            print(f"trace: {res.instructions_and_trace[1]}")
    out = np.stack([res.results[b]["out"] for b in range(B)], axis=0)
    return out.astype(np.float32)
